# revision 10
# baseline (speedup 1.0000x reference)
"""Trainium2 Bass kernel for the histogram-binning KL loss.

Strategy (v2)
-------------
 * Matrix-parallel sharding: cores 0-3 process T_F, cores 4-7 process S_F,
   64 cosine rows (16384 pairs) per core.  The program is SPMD-identical;
   only the per-core input tensors differ (which matrix, which row slice,
   which mask rows).  This halves the per-matrix front-end (load, normalize,
   transpose, Gram) relative to the row-only sharding.
 * Gaussian soft-binning on a 61-point coarse grid (18x decimation of the
   1000 fine bins); full histograms recovered by 6-point Lagrange
   interpolation.  A TensorE matmul produces q = 100 t d + ind_w
   (-50 d^2 + ln w); ScalarE evaluates exp(q - 50 t^2) via its per-partition
   bias and its fused accum register IS the weighted histogram partial.
 * The q matmul runs in bf16 with 2-term split-precision operands (hi/lo
   rows, K=7).  The dropped lo*lo cross terms land at ~1e-3 in q for the
   shared t-table (cancels between T and S in the KL ln-difference) and
   ~5e-4 per-pair otherwise -- far inside the 2e-2 gate.
 * ONE AllGather replaces the two serialized AllReduces: each core
   contributes [coarse hist (128) | E_pos rows (64) | E_neg rows (64) |
   o3 partial (1)].  Every core then combines the 8 slots locally
   (transpose + free-axis reduce), interpolates all four fine histograms
   in a single matmul, and evaluates the KL + order terms on device.

Host work is limited to argmax/label-mask construction and constant tables.
"""

import os
from contextlib import ExitStack

import ml_dtypes
import numpy as np

import concourse.bass as bass
import concourse.bacc as bacc
import concourse.tile as tile
from concourse import masks, mybir
from concourse.bass_utils import run_bass_kernel_spmd

F32 = mybir.dt.float32
BF16 = mybir.dt.bfloat16
NPBF = ml_dtypes.bfloat16
AF = mybir.ActivationFunctionType

N, D, C = 256, 512, 16
N_CORES = 8
GCORES = 4                     # cores per matrix
ROWS = N // GCORES             # 64 cosine rows per core
PAIRS = ROWS * N               # 16384 pair distances per core
S = 18                         # fine bins per coarse bin
ORDER = 6                      # Lagrange interpolation order
MC = (1000 + S - 1) // S + ORDER - 1   # 61 coarse bins
HALF = 64                      # partition half (pos rows 0:64, neg 64:128)
KQ = 7                         # split-bf16 contraction rows of the q matmul
BLK = 512                      # pairs per matmul (one PSUM region)
GRP = 1024                     # pairs per exp pass (2 blocks)
NGRP = PAIRS // GRP            # 16
NB = 1000
NBP = 1024                     # padded fine bins (zero tail)
EPS = 1e-9
INV2S2 = 50.0                  # 1 / (2 sigma^2)
LOG_ZERO = -60000.0            # ln(0) stand-in; exp underflows to exactly 0
PLEN = 2 * HALF + 2 * ROWS + 1         # 257 payload floats per core


def _bfsplit(x, n=2):
    """Split x into n bf16 terms summing to ~x (exact bf16 values)."""
    out, r = [], np.asarray(x, np.float64)
    for _ in range(n):
        h = r.astype(NPBF)
        out.append(h)
        r = r - h.astype(np.float64)
    return out


def _coarse_centers():
    m = np.arange(HALF, dtype=np.float64)
    return -1.0 + (0.002 * S) * (m - 1.0)   # entries >= MC are padding


def _rq_table():
    t = _coarse_centers()
    t100 = 2 * INV2S2 * t
    t100[MC:] = 0.0
    th, tl = _bfsplit(np.concatenate([t100, t100]), 2)
    indp = np.zeros(2 * HALF, NPBF)
    indp[:MC] = 1
    indn = np.zeros(2 * HALF, NPBF)
    indn[HALF : HALF + MC] = 1
    # row k of lhsT pairs with row k of the stitched rhs:
    # rhs rows [dh dl dh sph spl snh snl]
    return np.stack([th, th, tl, indp, indp, indn, indn]).astype(NPBF)


def _bq_table():
    t = _coarse_centers()
    bq = np.concatenate([-INV2S2 * t * t, -INV2S2 * t * t])[:, None]
    bq[MC:HALF] = LOG_ZERO
    bq[HALF + MC :] = LOG_ZERO
    return bq.astype(np.float32)


def _interp_table():
    wi = np.zeros((HALF, NBP), np.float64)
    nodes = np.arange(ORDER) - 1.0
    for r in range(S):
        x = r / S
        c = [
            np.prod([(x - nodes[j]) / (nodes[m] - nodes[j]) for j in range(ORDER) if j != m])
            for m in range(ORDER)
        ]
        ks = np.arange((NB - r + S - 1) // S)
        for m in range(ORDER):
            wi[ks + m, S * ks + r] = c[m]
    return np.vstack([wi, wi]).astype(np.float32)   # [128, NBP]


def build_nc():
    nc = bacc.Bacc(
        "TRN2", target_bir_lowering=False, debug=False, num_devices=N_CORES
    )

    xd = nc.dram_tensor("x", [N, D], F32, kind="ExternalInput")
    xrd = nc.dram_tensor("xr", [ROWS, D], F32, kind="ExternalInput")
    LPd = nc.dram_tensor("LP", [ROWS, N], F32, kind="ExternalInput")
    LNd = nc.dram_tensor("LN", [ROWS, N], F32, kind="ExternalInput")
    MPd = nc.dram_tensor("MP", [ROWS, N], F32, kind="ExternalInput")
    MNd = nc.dram_tensor("MN", [ROWS, N], F32, kind="ExternalInput")
    Rqd = nc.dram_tensor("Rq", [KQ, 2 * HALF], BF16, kind="ExternalInput")
    Bqd = nc.dram_tensor("Bq", [2 * HALF, 1], F32, kind="ExternalInput")
    WId = nc.dram_tensor("WI", [2 * HALF, NBP], F32, kind="ExternalInput")
    SLd = nc.dram_tensor("SL", [N_CORES, 1], F32, kind="ExternalInput")
    KCd = nc.dram_tensor("KC", [2, 1], F32, kind="ExternalInput")
    outd = nc.dram_tensor("out", [1, 1], F32, kind="ExternalOutput")

    with tile.TileContext(nc) as tc, ExitStack() as ctx:
        cpool = ctx.enter_context(tc.tile_pool(name="const", bufs=1))
        spool = ctx.enter_context(tc.tile_pool(name="stitch", bufs=1))
        xpool = ctx.enter_context(tc.tile_pool(name="x", bufs=1))
        tpool = ctx.enter_context(tc.tile_pool(name="xnt", bufs=2))
        qpool = ctx.enter_context(tc.tile_pool(name="q", bufs=2, space="PSUM"))
        ppool = ctx.enter_context(tc.tile_pool(name="pt", bufs=2, space="PSUM"))
        mpool = ctx.enter_context(tc.tile_pool(name="misc", bufs=2))
        rpool = ctx.enter_context(tc.tile_pool(name="res", bufs=1))
        drpool = ctx.enter_context(tc.tile_pool(name="dram", bufs=1, space="DRAM"))

        ident = cpool.tile([128, 128], F32)
        masks.make_identity(nc, ident[:])
        Rq = cpool.tile([KQ, 2 * HALF], BF16)
        nc.sync.dma_start(Rq[:], Rqd[:, :])
        Bq = cpool.tile([2 * HALF, 1], F32)
        nc.sync.dma_start(Bq[:], Bqd[:, :])
        LP = cpool.tile([ROWS, N], F32)
        nc.sync.dma_start(LP[:], LPd[:, :])
        LNt = cpool.tile([ROWS, N], F32)
        nc.sync.dma_start(LNt[:], LNd[:, :])
        MP = cpool.tile([ROWS, N], F32)
        nc.sync.dma_start(MP[:], MPd[:, :])
        MN = cpool.tile([ROWS, N], F32)
        nc.sync.dma_start(MN[:], MNd[:, :])
        WI = cpool.tile([2 * HALF, NBP], F32)
        nc.sync.dma_start(WI[:], WId[:, :])
        SL2 = cpool.tile([N_CORES, 1], F32)
        nc.sync.dma_start(SL2[:], SLd[:, :])
        kcoef = cpool.tile([2, 1], F32)
        nc.sync.dma_start(kcoef[:], KCd[:, :])
        ones = cpool.tile([128, 1], F32)
        nc.vector.memset(ones[:], 1.0)

        cc_in = drpool.tile([PLEN, 1], F32)
        cc_out = drpool.tile([N_CORES * PLEN, 1], F32, addr_space="Shared")

        # ---- load + row-normalize the full matrix and this core's slice
        xn_t = []
        for h in range(2):
            xa = xpool.tile([128, D], F32, tag=f"xa{h}")
            nc.sync.dma_start(xa[:], xd[128 * h : 128 * (h + 1), :])
            junk = xpool.tile([128, D], F32, tag="junk")
            nrm2 = mpool.tile([128, 1], F32, tag="nrm2")
            nc.vector.scalar_tensor_tensor(
                junk[:], xa[:], 1.0, xa[:],
                mybir.AluOpType.bypass, mybir.AluOpType.mult,
                accum_out=nrm2[:],
            )
            srt = mpool.tile([128, 1], F32, tag="srt")
            nc.scalar.activation(srt[:], nrm2[:], AF.Sqrt)
            rn = mpool.tile([128, 1], F32, tag="rn")
            nc.vector.reciprocal(rn[:], srt[:])
            xn = xpool.tile([128, D], F32, tag=f"xn{h}")
            nc.vector.tensor_scalar_mul(xn[:], xa[:], rn[:])
            xn_t.append(xn)

        xra = xpool.tile([ROWS, D], F32, tag="xra")
        nc.sync.dma_start(xra[:], xrd[:, :])
        junkr = xpool.tile([ROWS, D], F32, tag="junkr")
        nrm2r = mpool.tile([ROWS, 1], F32, tag="nrm2r")
        nc.vector.scalar_tensor_tensor(
            junkr[:], xra[:], 1.0, xra[:],
            mybir.AluOpType.bypass, mybir.AluOpType.mult,
            accum_out=nrm2r[:],
        )
        srtr = mpool.tile([ROWS, 1], F32, tag="srtr")
        nc.scalar.activation(srtr[:], nrm2r[:], AF.Sqrt)
        rnr = mpool.tile([ROWS, 1], F32, tag="rnr")
        nc.vector.reciprocal(rnr[:], srtr[:])
        xnr = xpool.tile([ROWS, D], F32, tag="xnr")
        nc.vector.tensor_scalar_mul(xnr[:], xra[:], rnr[:])

        # ---- transpose xn (full) and xnr (slice) into d-major layout,
        #      accumulating the cos Gram slice chunk by chunk
        cps = ppool.tile([ROWS, N], F32, tag="cos_ps", bufs=1)
        for c in range(4):
            xt = tpool.tile([128, N], F32, tag="xnT")
            for h in range(2):
                pt = ppool.tile([128, 128], F32, tag="ps_t")
                nc.tensor.transpose(
                    pt[:], xn_t[h][:, 128 * c : 128 * (c + 1)], ident[:]
                )
                nc.vector.tensor_copy(xt[:, 128 * h : 128 * (h + 1)], pt[:])
            ptr = ppool.tile([128, ROWS], F32, tag="ps_t")
            nc.tensor.transpose(
                ptr[:], xnr[:, 128 * c : 128 * (c + 1)], ident[:ROWS, :ROWS]
            )
            xtr = tpool.tile([128, ROWS], F32, tag="xnrT")
            nc.vector.tensor_copy(xtr[:], ptr[:])
            nc.tensor.matmul(
                cps[:], xtr[:], xt[:], start=(c == 0), stop=(c == 3)
            )
        cos_sb = mpool.tile([ROWS, N], F32, tag="cos_sb")
        nc.vector.tensor_copy(cos_sb[:], cps[:])

        # ---- E columns (weighted row means of cos) + o3 partial
        e2 = rpool.tile([ROWS, 2], F32)
        junkE = mpool.tile([ROWS, N], F32, tag="junkE")
        for col, msk in ((0, MP), (1, MN)):
            nc.vector.scalar_tensor_tensor(
                junkE[:], cos_sb[:], 1.0, msk[:],
                mybir.AluOpType.bypass, mybir.AluOpType.mult,
                accum_out=e2[:, col : col + 1],
            )
        ed = rpool.tile([ROWS, 1], F32)
        nc.vector.tensor_sub(ed[:], e2[:, 0:1], e2[:, 1:2])
        o3_ps = ppool.tile([1, 3], F32, tag="ps_s", bufs=1)
        nc.tensor.matmul(o3_ps[:, 0:1], ones[:ROWS, :], ed[:], start=True, stop=True)
        o3_sb = rpool.tile([1, 1], F32)
        nc.vector.tensor_copy(o3_sb[:], o3_ps[:, 0:1])
        nc.sync.dma_start(cc_in[2 * HALF + 2 * ROWS :, 0:1], o3_sb[:])
        nc.sync.dma_start(cc_in[2 * HALF : 2 * HALF + ROWS, 0:1], e2[:, 0:1])
        nc.sync.dma_start(cc_in[2 * HALF + ROWS : 2 * HALF + 2 * ROWS, 0:1], e2[:, 1:2])

        # ---- split-bf16 component blocks of one wide tile, then stitch
        sb7 = spool.tile([ROWS, KQ * N], BF16)

        def blk_(r):
            return sb7[:, N * r : N * (r + 1)]

        sq_sb = mpool.tile([ROWS, N], F32, tag="sq_sb")
        nc.vector.tensor_mul(sq_sb[:], cos_sb[:], cos_sb[:])
        spn_f = mpool.tile([ROWS, N], F32, tag="spn")
        nc.vector.scalar_tensor_tensor(
            spn_f[:], sq_sb[:], -INV2S2, LP[:],
            mybir.AluOpType.mult, mybir.AluOpType.add,
        )
        snn_f = mpool.tile([ROWS, N], F32, tag="snn")
        nc.vector.scalar_tensor_tensor(
            snn_f[:], sq_sb[:], -INV2S2, LNt[:],
            mybir.AluOpType.mult, mybir.AluOpType.add,
        )

        nc.scalar.copy(blk_(0), cos_sb[:])            # dh
        t1_f = mpool.tile([ROWS, N], F32, tag="t1")
        nc.vector.tensor_sub(t1_f[:], cos_sb[:], blk_(0))
        nc.vector.tensor_copy(blk_(1), t1_f[:])       # dl
        nc.scalar.copy(blk_(2), blk_(0))              # dh (dup for tl row)
        nc.scalar.copy(blk_(3), spn_f[:])             # sph
        nc.vector.tensor_sub(blk_(4), spn_f[:], blk_(3))   # spl
        nc.scalar.copy(blk_(5), snn_f[:])             # snh
        nc.vector.tensor_sub(blk_(6), snn_f[:], blk_(5))   # snl

        st = spool.tile([KQ, PAIRS], BF16)
        for r in range(KQ):
            nc.sync.dma_start(
                st[r : r + 1, :].rearrange("p (r c) -> p r c", r=ROWS),
                blk_(r),
            )

        # ---- main loop: q matmul -> exp with fused histogram accum
        hacc = rpool.tile([128, NGRP], F32)
        djunk = mpool.tile([128, GRP], F32, tag="djunk", bufs=1)
        for g in range(NGRP):
            q2 = qpool.tile([128, GRP], F32, tag="q2")
            for b in range(GRP // BLK):
                lo = GRP * g + BLK * b
                nc.tensor.matmul(
                    q2[:, BLK * b : BLK * (b + 1)],
                    Rq[:],
                    st[:, lo : lo + BLK],
                    start=True,
                    stop=True,
                )
            nc.scalar.activation(
                djunk[:], q2[:], AF.Exp, bias=Bq[:],
                accum_out=hacc[:, g : g + 1],
            )
        hcol = rpool.tile([128, 1], F32)
        nc.vector.reduce_sum(hcol[:], hacc[:], axis=mybir.AxisListType.X)
        nc.sync.dma_start(cc_in[0 : 2 * HALF, 0:1], hcol[:])

        # ---- ONE AllGather of every core's [hist | E_pos | E_neg | o3]
        nc.gpsimd.collective_compute(
            "AllGather",
            mybir.AluOpType.bypass,
            replica_groups=[list(range(N_CORES))],
            ins=[cc_in[:, :].opt()],
            outs=[cc_out[:, :].opt()],
        )

        # ---- local combine: coarse hists, order terms
        G = rpool.tile([N_CORES, PLEN], F32)
        nc.sync.dma_start(
            G[:], cc_out[:, 0:1].rearrange("(s c) w -> s (c w)", s=N_CORES)
        )
        ght = ppool.tile([128, N_CORES], F32, tag="ps_t")
        nc.tensor.transpose(ght[:], G[:, 0 : 2 * HALF], ident[:N_CORES, :N_CORES])
        htt = rpool.tile([128, 2], F32)
        nc.vector.reduce_sum(htt[:, 0:1], ght[:, 0:GCORES], axis=mybir.AxisListType.X)
        nc.vector.reduce_sum(htt[:, 1:2], ght[:, GCORES:], axis=mybir.AxisListType.X)
        # interp lhsT: cols 0,1 = T pos/neg; cols 32,33 = S pos/neg (the S
        # fine hists land on partition 32 so activations stay quad-aligned)
        lh4 = rpool.tile([128, 34], F32)
        nc.vector.memset(lh4[:], 0.0)
        nc.vector.tensor_copy(lh4[0:HALF, 0:1], htt[0:HALF, 0:1])
        nc.vector.tensor_copy(lh4[HALF:, 1:2], htt[HALF:, 0:1])
        nc.vector.tensor_copy(lh4[0:HALF, 32:33], htt[0:HALF, 1:2])
        nc.vector.tensor_copy(lh4[HALF:, 33:34], htt[HALF:, 1:2])

        get = ppool.tile([128, N_CORES], F32, tag="ps_t")
        nc.tensor.transpose(
            get[:], G[:, 2 * HALF : 2 * HALF + 2 * ROWS],
            ident[:N_CORES, :N_CORES],
        )
        gE = rpool.tile([128, N_CORES], F32)
        nc.vector.tensor_copy(gE[:], get[:])
        dE = rpool.tile([128, GCORES], F32)
        nc.vector.tensor_sub(dE[:], gE[:, 0:GCORES], gE[:, GCORES:])
        aE = rpool.tile([128, GCORES], F32)
        acol = rpool.tile([128, 1], F32)
        nc.scalar.activation(aE[:], dE[:], AF.Abs, accum_out=acol[:])
        ps3 = ppool.tile([1, 3], F32, tag="ps_s", bufs=1)
        nc.tensor.matmul(ps3[:, 0:1], ones[:], acol[:], start=True, stop=True)
        nc.tensor.matmul(
            ps3[:, 1:2], SL2[:], G[:, PLEN - 1 : PLEN], start=True, stop=True
        )

        # ---- interpolate all four fine hists in one matmul, KL terms
        fh = qpool.tile([128, NBP], F32, tag="q2")
        for half in range(2):
            cols = slice(512 * half, 512 * (half + 1))
            nc.tensor.matmul(
                fh[0:34, cols], lh4[:], WI[:, cols], start=True, stop=True
            )
        avT = rpool.tile([2, NBP], F32)
        nc.vector.tensor_scalar(
            avT[:], fh[0:2, :], 0.0, EPS,
            mybir.AluOpType.max, mybir.AluOpType.add,
        )
        avS = rpool.tile([2, NBP], F32)
        nc.vector.tensor_scalar(
            avS[:], fh[32:34, :], 0.0, EPS,
            mybir.AluOpType.max, mybir.AluOpType.add,
        )
        lnT = rpool.tile([2, NBP], F32)
        nc.scalar.activation(lnT[:], avT[:], AF.Ln)
        lnS = rpool.tile([2, NBP], F32)
        nc.scalar.activation(lnS[:], avS[:], AF.Ln)
        dif = rpool.tile([2, NBP], F32)
        nc.vector.tensor_sub(dif[:], lnT[:], lnS[:])
        junkk = rpool.tile([2, NBP], F32)
        kl2 = rpool.tile([2, 1], F32)
        nc.vector.scalar_tensor_tensor(
            junkk[:], avT[:], 1.0, dif[:],
            mybir.AluOpType.bypass, mybir.AluOpType.mult,
            accum_out=kl2[:],
        )
        nc.tensor.matmul(ps3[:, 2:3], kcoef[:], kl2[:], start=True, stop=True)

        # ---- final scalar: kl + 0.5 * (o12 + o3) / N
        sc3 = rpool.tile([1, 3], F32)
        nc.vector.tensor_copy(sc3[:], ps3[:])
        osum = rpool.tile([1, 1], F32)
        nc.vector.tensor_add(osum[:], sc3[:, 0:1], sc3[:, 1:2])
        fin = rpool.tile([1, 1], F32)
        nc.vector.scalar_tensor_tensor(
            fin[:], osum[:], 0.5 / N, sc3[:, 2:3],
            mybir.AluOpType.mult, mybir.AluOpType.add,
        )
        nc.sync.dma_start(outd[:, :], fin[:])

    nc.compile()
    return nc


def _host_inputs(T_F, S_F, labels):
    T_F = np.ascontiguousarray(T_F, np.float32)
    S_F = np.ascontiguousarray(S_F, np.float32)
    labels = np.asarray(labels)
    lab = np.argmax(labels, axis=-1)
    grid = (lab[None, :] == lab[:, None]).astype(np.float32)
    neg_l = 1.0 - grid
    pos_l = grid * (1.0 - np.eye(N, dtype=np.float32))
    pw = pos_l / pos_l.sum()
    nw = neg_l / neg_l.sum()
    lpw = np.full_like(pw, LOG_ZERO)
    np.log(pw, out=lpw, where=pw > 0)
    lnw = np.full_like(nw, LOG_ZERO)
    np.log(nw, out=lnw, where=nw > 0)
    mp = pos_l / pos_l.sum(-1, keepdims=True)
    mn = neg_l / neg_l.sum(-1, keepdims=True)

    rq = _rq_table()
    bq = _bq_table()
    wi = _interp_table()
    kc = np.array([[0.1], [0.02]], np.float32)
    sl = np.array([[0.0]] * GCORES + [[1.0]] * GCORES, np.float32)

    in_maps = []
    for c in range(N_CORES):
        mat = T_F if c < GCORES else S_F
        rows = slice(ROWS * (c % GCORES), ROWS * (c % GCORES + 1))
        in_maps.append(
            {
                "x": mat,
                "xr": np.ascontiguousarray(mat[rows]),
                "LP": np.ascontiguousarray(lpw[rows].astype(np.float32)),
                "LN": np.ascontiguousarray(lnw[rows].astype(np.float32)),
                "MP": np.ascontiguousarray(mp[rows].astype(np.float32)),
                "MN": np.ascontiguousarray(mn[rows].astype(np.float32)),
                "Rq": rq,
                "Bq": bq,
                "WI": wi,
                "SL": sl,
                "KC": kc,
            }
        )
    return in_maps


_NC_CACHE = {}


def run(T_F, S_F, labels, trace=False):
    if "nc" not in _NC_CACHE:
        _NC_CACHE["nc"] = build_nc()
    nc = _NC_CACHE["nc"]
    in_maps = _host_inputs(T_F, S_F, labels)
    res = run_bass_kernel_spmd(
        nc, in_maps, core_ids=list(range(N_CORES)), trace=trace
    )
    val = np.float32(res.results[0]["out"][0, 0])
    return val, res


def kernel(T_F, S_F, labels):
    val, _ = run(T_F, S_F, labels)
    return np.array(val, dtype=np.float32)


# revision 20
# speedup vs baseline: 1.0777x; 1.0777x over previous
"""Trainium2 Bass kernel for the histogram-binning KL loss.

Strategy (v2)
-------------
 * Matrix-parallel sharding: cores 0-3 process T_F, cores 4-7 process S_F,
   64 cosine rows (16384 pairs) per core.  The program is SPMD-identical;
   only the per-core input tensors differ (which matrix, which row slice,
   which mask rows).  This halves the per-matrix front-end (load, normalize,
   transpose, Gram) relative to the row-only sharding.
 * Gaussian soft-binning on a 61-point coarse grid (18x decimation of the
   1000 fine bins); full histograms recovered by 6-point Lagrange
   interpolation.  A TensorE matmul produces q = 100 t d + ind_w
   (-50 d^2 + ln w); ScalarE evaluates exp(q - 50 t^2) via its per-partition
   bias and its fused accum register IS the weighted histogram partial.
 * The q matmul runs in bf16 with 2-term split-precision operands (hi/lo
   rows, K=7).  The dropped lo*lo cross terms land at ~1e-3 in q for the
   shared t-table (cancels between T and S in the KL ln-difference) and
   ~5e-4 per-pair otherwise -- far inside the 2e-2 gate.
 * ONE AllGather replaces the two serialized AllReduces: each core
   contributes [coarse hist (128) | E_pos rows (64) | E_neg rows (64) |
   o3 partial (1)].  Every core then combines the 8 slots locally
   (transpose + free-axis reduce), interpolates all four fine histograms
   in a single matmul, and evaluates the KL + order terms on device.

Host work is limited to argmax/label-mask construction and constant tables.
"""

import os
from contextlib import ExitStack

import ml_dtypes
import numpy as np

import concourse.bass as bass
import concourse.bacc as bacc
import concourse.tile as tile
from concourse import masks, mybir
from concourse.bass_utils import run_bass_kernel_spmd

F32 = mybir.dt.float32
BF16 = mybir.dt.bfloat16
NPBF = ml_dtypes.bfloat16
AF = mybir.ActivationFunctionType

N, D, C = 256, 512, 16
N_CORES = 8
GCORES = 4                     # cores per matrix
ROWS = N // GCORES             # 64 cosine rows per core
PAIRS = ROWS * N               # 16384 pair distances per core
S = 18                         # fine bins per coarse bin
ORDER = 6                      # Lagrange interpolation order
MC = (1000 + S - 1) // S + ORDER - 1   # 61 coarse bins
HALF = 64                      # partition half (pos rows 0:64, neg 64:128)
KQ = 7                         # split-bf16 contraction rows of the q matmul
BLK = 512                      # pairs per matmul (one PSUM region)
GRP = 1024                     # pairs per exp pass (2 blocks)
NGRP = PAIRS // GRP            # 16
NB = 1000
NBP = 1024                     # padded fine bins (zero tail)
EPS = 1e-9
INV2S2 = 50.0                  # 1 / (2 sigma^2)
LOG_ZERO = -60000.0            # ln(0) stand-in; exp underflows to exactly 0
PLEN = 2 * 2 * HALF + 4 * N + 1        # 1281 payload floats (position-coded)


def _bfsplit(x, n=2):
    """Split x into n bf16 terms summing to ~x (exact bf16 values)."""
    out, r = [], np.asarray(x, np.float64)
    for _ in range(n):
        h = r.astype(NPBF)
        out.append(h)
        r = r - h.astype(np.float64)
    return out


def _coarse_centers():
    m = np.arange(HALF, dtype=np.float64)
    return -1.0 + (0.002 * S) * (m - 1.0)   # entries >= MC are padding


def _rq_table():
    t = _coarse_centers()
    t100 = 2 * INV2S2 * t
    t100[MC:] = 0.0
    th, tl = _bfsplit(np.concatenate([t100, t100]), 2)
    indp = np.zeros(2 * HALF, NPBF)
    indp[:MC] = 1
    indn = np.zeros(2 * HALF, NPBF)
    indn[HALF : HALF + MC] = 1
    # row k of lhsT pairs with row k of the stitched rhs:
    # rhs rows [dh dl dh sph spl snh snl]
    return np.stack([th, th, tl, indp, indp, indn, indn]).astype(NPBF)


def _bq_table():
    t = _coarse_centers()
    bq = np.concatenate([-INV2S2 * t * t, -INV2S2 * t * t])[:, None]
    bq[MC:HALF] = LOG_ZERO
    bq[HALF + MC :] = LOG_ZERO
    return bq.astype(np.float32)


def _interp_table():
    wi = np.zeros((HALF, NBP), np.float64)
    nodes = np.arange(ORDER) - 1.0
    for r in range(S):
        x = r / S
        c = [
            np.prod([(x - nodes[j]) / (nodes[m] - nodes[j]) for j in range(ORDER) if j != m])
            for m in range(ORDER)
        ]
        ks = np.arange((NB - r + S - 1) // S)
        for m in range(ORDER):
            wi[ks + m, S * ks + r] = c[m]
    return np.vstack([wi, wi]).astype(np.float32)   # [128, NBP]


def build_nc():
    nc = bacc.Bacc(
        "TRN2", target_bir_lowering=False, debug=False, num_devices=N_CORES
    )

    xd = nc.dram_tensor("x", [N, D], F32, kind="ExternalInput")
    xrd = nc.dram_tensor("xr", [ROWS, D], F32, kind="ExternalInput")
    LPd = nc.dram_tensor("LP", [ROWS, N], F32, kind="ExternalInput")
    LNd = nc.dram_tensor("LN", [ROWS, N], F32, kind="ExternalInput")
    MPd = nc.dram_tensor("MP", [ROWS, N], F32, kind="ExternalInput")
    MNd = nc.dram_tensor("MN", [ROWS, N], F32, kind="ExternalInput")
    Rqd = nc.dram_tensor("Rq", [KQ, 2 * HALF], BF16, kind="ExternalInput")
    Bqd = nc.dram_tensor("Bq", [2 * HALF, 1], F32, kind="ExternalInput")
    WId = nc.dram_tensor("WI", [2 * HALF, NBP], F32, kind="ExternalInput")
    M2d = nc.dram_tensor("M2", [2 * HALF, 2], F32, kind="ExternalInput")
    Pd = nc.dram_tensor("P", [ROWS, N], F32, kind="ExternalInput")
    KCd = nc.dram_tensor("KC", [2, 1], F32, kind="ExternalInput")
    outd = nc.dram_tensor("out", [1, 1], F32, kind="ExternalOutput")

    with tile.TileContext(nc) as tc, ExitStack() as ctx:
        cpool = ctx.enter_context(tc.tile_pool(name="const", bufs=1))
        spool = ctx.enter_context(tc.tile_pool(name="stitch", bufs=1))
        xpool = ctx.enter_context(tc.tile_pool(name="x", bufs=1))
        tpool = ctx.enter_context(tc.tile_pool(name="xnt", bufs=2))
        qpool = ctx.enter_context(tc.tile_pool(name="q", bufs=2, space="PSUM"))
        ppool = ctx.enter_context(tc.tile_pool(name="pt", bufs=2, space="PSUM"))
        mpool = ctx.enter_context(tc.tile_pool(name="misc", bufs=2))
        rpool = ctx.enter_context(tc.tile_pool(name="res", bufs=1))
        drpool = ctx.enter_context(tc.tile_pool(name="dram", bufs=1, space="DRAM"))

        # data DMAs first: the x/xr loads gate the whole front-end chain
        xa_t = []
        for h in range(2):
            xa = xpool.tile([128, D], F32, tag=f"xa{h}")
            nc.sync.dma_start(xa[:], xd[128 * h : 128 * (h + 1), :])
            xa_t.append(xa)
        xra = xpool.tile([ROWS, D], F32, tag="xra")
        nc.sync.dma_start(xra[:], xrd[:, :])

        ident = cpool.tile([128, 128], F32)
        masks.make_identity(nc, ident[:])
        LP = cpool.tile([ROWS, N], F32)
        nc.sync.dma_start(LP[:], LPd[:, :])
        LNt = cpool.tile([ROWS, N], F32)
        nc.sync.dma_start(LNt[:], LNd[:, :])
        MP = cpool.tile([ROWS, N], F32)
        nc.sync.dma_start(MP[:], MPd[:, :])
        MN = cpool.tile([ROWS, N], F32)
        nc.sync.dma_start(MN[:], MNd[:, :])
        Rq = cpool.tile([KQ, 2 * HALF], BF16)
        nc.sync.dma_start(Rq[:], Rqd[:, :])
        Bq = cpool.tile([2 * HALF, 1], F32)
        nc.sync.dma_start(Bq[:], Bqd[:, :])
        M2 = cpool.tile([2 * HALF, 2], F32)
        nc.sync.dma_start(M2[:], M2d[:, :])
        Pm = cpool.tile([ROWS, N], F32)
        nc.sync.dma_start(Pm[:], Pd[:, :])
        WI = cpool.tile([2 * HALF, NBP], F32)
        nc.sync.dma_start(WI[:], WId[:, :])
        kcoef = cpool.tile([2, 1], F32)
        nc.sync.dma_start(kcoef[:], KCd[:, :])
        ones = cpool.tile([128, 1], F32)
        nc.vector.memset(ones[:], 1.0)

        cc_in = drpool.tile([PLEN, 1], F32)
        cc_out = drpool.tile([PLEN, 1], F32, addr_space="Shared")

        # ---- row-normalize the full matrix and this core's slice
        xn_t = []
        for h in range(2):
            xa = xa_t[h]
            junk = xpool.tile([128, D], F32, tag="junk")
            nrm2 = mpool.tile([128, 1], F32, tag="nrm2")
            nc.vector.scalar_tensor_tensor(
                junk[:], xa[:], 1.0, xa[:],
                mybir.AluOpType.bypass, mybir.AluOpType.mult,
                accum_out=nrm2[:],
            )
            srt = mpool.tile([128, 1], F32, tag="srt")
            nc.scalar.activation(srt[:], nrm2[:], AF.Sqrt)
            rn = mpool.tile([128, 1], F32, tag="rn")
            nc.vector.reciprocal(rn[:], srt[:])
            xn = xpool.tile([128, D], F32, tag=f"xn{h}")
            nc.vector.tensor_scalar_mul(xn[:], xa[:], rn[:])
            xn_t.append(xn)

        junkr = xpool.tile([ROWS, D], F32, tag="junkr")
        nrm2r = mpool.tile([ROWS, 1], F32, tag="nrm2r")
        nc.vector.scalar_tensor_tensor(
            junkr[:], xra[:], 1.0, xra[:],
            mybir.AluOpType.bypass, mybir.AluOpType.mult,
            accum_out=nrm2r[:],
        )
        srtr = mpool.tile([ROWS, 1], F32, tag="srtr")
        nc.scalar.activation(srtr[:], nrm2r[:], AF.Sqrt)
        rnr = mpool.tile([ROWS, 1], F32, tag="rnr")
        nc.vector.reciprocal(rnr[:], srtr[:])
        xnr = xpool.tile([ROWS, D], F32, tag="xnr")
        nc.vector.tensor_scalar_mul(xnr[:], xra[:], rnr[:])

        # ---- transpose xn (full) and xnr (slice) into d-major layout,
        #      accumulating the cos Gram slice chunk by chunk
        cps = ppool.tile([ROWS, N], F32, tag="cos_ps", bufs=1)
        for c in range(4):
            xt = tpool.tile([128, N], F32, tag="xnT")
            for h in range(2):
                pt = ppool.tile([128, 128], F32, tag="ps_t")
                nc.tensor.transpose(
                    pt[:], xn_t[h][:, 128 * c : 128 * (c + 1)], ident[:]
                )
                nc.vector.tensor_copy(xt[:, 128 * h : 128 * (h + 1)], pt[:])
            ptr = ppool.tile([128, ROWS], F32, tag="ps_t")
            nc.tensor.transpose(
                ptr[:], xnr[:, 128 * c : 128 * (c + 1)], ident[:ROWS, :ROWS]
            )
            xtr = tpool.tile([128, ROWS], F32, tag="xnrT")
            nc.vector.tensor_copy(xtr[:], ptr[:])
            nc.tensor.matmul(
                cps[:], xtr[:], xt[:], start=(c == 0), stop=(c == 3)
            )
        cos_sb = mpool.tile([ROWS, N], F32, tag="cos_sb")
        nc.vector.tensor_copy(cos_sb[:], cps[:])

        # ---- E columns (weighted row means of cos) + o3 partial
        e2 = rpool.tile([ROWS, 2], F32)
        junkE = mpool.tile([ROWS, N], F32, tag="junkE")
        for col, msk in ((0, MP), (1, MN)):
            nc.vector.scalar_tensor_tensor(
                junkE[:], cos_sb[:], 1.0, msk[:],
                mybir.AluOpType.bypass, mybir.AluOpType.mult,
                accum_out=e2[:, col : col + 1],
            )
        # mask E by matrix (cols: EposT EnegT EposS EnegS) and scatter the
        # 64 rows into their global [256] positions with one P matmul
        ed = rpool.tile([ROWS, 1], F32)
        nc.vector.tensor_sub(ed[:], e2[:, 0:1], e2[:, 1:2])
        o3_ps = ppool.tile([1, 3], F32, tag="ps_s", bufs=1)
        nc.tensor.matmul(
            o3_ps[:, 0:1], M2[0:ROWS, 1:2], ed[:], start=True, stop=True
        )
        o3_sb = rpool.tile([1, 1], F32)
        nc.vector.tensor_copy(o3_sb[:], o3_ps[:, 0:1])
        nc.sync.dma_start(cc_in[PLEN - 1 :, 0:1], o3_sb[:])
        e4 = rpool.tile([ROWS, 4], F32)
        nc.vector.tensor_scalar_mul(e4[:, 0:2], e2[:], M2[0:ROWS, 0:1])
        nc.vector.tensor_scalar_mul(e4[:, 2:4], e2[:], M2[0:ROWS, 1:2])
        ep_ps = ppool.tile([4, N], F32, tag="cos_ps", bufs=1)
        nc.tensor.matmul(ep_ps[:], e4[:], Pm[:], start=True, stop=True)
        ep_sb = rpool.tile([4, N], F32)
        nc.vector.tensor_copy(ep_sb[:], ep_ps[:])
        nc.sync.dma_start(
            cc_in[2 * 2 * HALF : 2 * 2 * HALF + 4 * N, 0:1].rearrange(
                "(k j) q -> k (j q)", k=4
            ),
            ep_sb[:],
        )

        # ---- split-bf16 component blocks of one wide tile, then stitch
        sb7 = spool.tile([ROWS, KQ * N], BF16)

        def blk_(r):
            return sb7[:, N * r : N * (r + 1)]

        sq_sb = mpool.tile([ROWS, N], F32, tag="sq_sb")
        nc.vector.tensor_mul(sq_sb[:], cos_sb[:], cos_sb[:])
        spn_f = mpool.tile([ROWS, N], F32, tag="spn")
        nc.vector.scalar_tensor_tensor(
            spn_f[:], sq_sb[:], -INV2S2, LP[:],
            mybir.AluOpType.mult, mybir.AluOpType.add,
        )
        snn_f = mpool.tile([ROWS, N], F32, tag="snn")
        nc.vector.scalar_tensor_tensor(
            snn_f[:], sq_sb[:], -INV2S2, LNt[:],
            mybir.AluOpType.mult, mybir.AluOpType.add,
        )

        nc.scalar.copy(blk_(0), cos_sb[:])            # dh
        t1_f = mpool.tile([ROWS, N], F32, tag="t1")
        nc.vector.tensor_sub(t1_f[:], cos_sb[:], blk_(0))
        nc.vector.tensor_copy(blk_(1), t1_f[:])       # dl
        nc.scalar.copy(blk_(2), blk_(0))              # dh (dup for tl row)
        nc.scalar.copy(blk_(3), spn_f[:])             # sph
        nc.vector.tensor_sub(blk_(4), spn_f[:], blk_(3))   # spl
        nc.scalar.copy(blk_(5), snn_f[:])             # snh
        nc.vector.tensor_sub(blk_(6), snn_f[:], blk_(5))   # snl

        st = spool.tile([KQ, PAIRS], BF16)
        for r in range(KQ):
            nc.sync.dma_start(
                st[r : r + 1, :].rearrange("p (r c) -> p r c", r=ROWS),
                blk_(r),
            )

        # ---- main loop: q matmul -> exp with fused histogram accum
        hacc = rpool.tile([128, NGRP], F32)
        djunk = mpool.tile([128, GRP], BF16, tag="djunk", bufs=1)
        for g in range(NGRP):
            q2 = qpool.tile([128, GRP], F32, tag="q2")
            for b in range(GRP // BLK):
                lo = GRP * g + BLK * b
                nc.tensor.matmul(
                    q2[:, BLK * b : BLK * (b + 1)],
                    Rq[:],
                    st[:, lo : lo + BLK],
                    start=True,
                    stop=True,
                )
            nc.scalar.activation(
                djunk[:], q2[:], AF.Exp, bias=Bq[:],
                accum_out=hacc[:, g : g + 1],
            )
        hcol = rpool.tile([128, 1], F32)
        nc.vector.reduce_sum(hcol[:], hacc[:], axis=mybir.AxisListType.X)
        hcol2 = rpool.tile([128, 2], F32)
        nc.vector.tensor_scalar_mul(hcol2[:], M2[:], hcol[:])
        nc.sync.dma_start(
            cc_in[0 : 2 * 2 * HALF, 0:1].rearrange("(w b) q -> b (w q)", w=2),
            hcol2[:],
        )

        # ---- ONE masked AllReduce: [histT(128) histS(128) E(4x256) o3]
        nc.gpsimd.collective_compute(
            "AllReduce",
            mybir.AluOpType.add,
            replica_groups=[list(range(N_CORES))],
            ins=[cc_in[:, :].opt()],
            outs=[cc_out[:, :].opt()],
        )

        # ---- local combine (cols: histT histS EposT(2) EnegT(2) EposS(2)
        #      EnegS(2)); no cross-partition work needed
        G = rpool.tile([128, 10], F32)
        nc.sync.dma_start(
            G[:], cc_out[0 : PLEN - 1, 0:1].rearrange("(m b) q -> b (m q)", m=10)
        )
        g3 = rpool.tile([1, 1], F32)
        nc.sync.dma_start(g3[:], cc_out[PLEN - 1 :, 0:1])
        # interp lhsT: cols 0,1 = T pos/neg; cols 32,33 = S pos/neg (the S
        # fine hists land on partition 32 so activations stay quad-aligned)
        lh4 = rpool.tile([128, 34], F32)
        nc.vector.memset(lh4[:], 0.0)
        nc.vector.tensor_copy(lh4[0:HALF, 0:1], G[0:HALF, 0:1])
        nc.vector.tensor_copy(lh4[HALF:, 1:2], G[HALF:, 0:1])
        nc.vector.tensor_copy(lh4[0:HALF, 32:33], G[0:HALF, 1:2])
        nc.vector.tensor_copy(lh4[HALF:, 33:34], G[HALF:, 1:2])

        dE = rpool.tile([128, 4], F32)
        nc.vector.tensor_sub(dE[:], G[:, 2:6], G[:, 6:10])
        aE = rpool.tile([128, 4], F32)
        acol = rpool.tile([128, 1], F32)
        nc.scalar.activation(aE[:], dE[:], AF.Abs, accum_out=acol[:])
        ps3 = ppool.tile([1, 3], F32, tag="ps_s", bufs=1)
        nc.tensor.matmul(ps3[:, 0:1], ones[:], acol[:], start=True, stop=True)

        # ---- interpolate all four fine hists in one matmul, KL terms
        fh = qpool.tile([128, NBP], F32, tag="q2")
        for half in range(2):
            cols = slice(512 * half, 512 * (half + 1))
            nc.tensor.matmul(
                fh[0:34, cols], lh4[:], WI[:, cols], start=True, stop=True
            )
        avT = rpool.tile([2, NBP], F32)
        nc.vector.tensor_scalar(
            avT[:], fh[0:2, :], 0.0, EPS,
            mybir.AluOpType.max, mybir.AluOpType.add,
        )
        avS = rpool.tile([2, NBP], F32)
        nc.vector.tensor_scalar(
            avS[:], fh[32:34, :], 0.0, EPS,
            mybir.AluOpType.max, mybir.AluOpType.add,
        )
        lnT = rpool.tile([2, NBP], F32)
        nc.scalar.activation(lnT[:], avT[:], AF.Ln)
        lnS = rpool.tile([2, NBP], F32)
        nc.scalar.activation(lnS[:], avS[:], AF.Ln)
        dif = rpool.tile([2, NBP], F32)
        nc.vector.tensor_sub(dif[:], lnT[:], lnS[:])
        junkk = rpool.tile([2, NBP], F32)
        kl2 = rpool.tile([2, 1], F32)
        nc.vector.scalar_tensor_tensor(
            junkk[:], avT[:], 1.0, dif[:],
            mybir.AluOpType.bypass, mybir.AluOpType.mult,
            accum_out=kl2[:],
        )
        nc.tensor.matmul(ps3[:, 2:3], kcoef[:], kl2[:], start=True, stop=True)

        # ---- final scalar: kl + 0.5 * (o12 + o3) / N
        sc3 = rpool.tile([1, 3], F32)
        nc.vector.tensor_copy(sc3[:], ps3[:])
        osum = rpool.tile([1, 1], F32)
        nc.vector.tensor_add(osum[:], sc3[:, 0:1], g3[:])
        fin = rpool.tile([1, 1], F32)
        nc.vector.scalar_tensor_tensor(
            fin[:], osum[:], 0.5 / N, sc3[:, 2:3],
            mybir.AluOpType.mult, mybir.AluOpType.add,
        )
        nc.sync.dma_start(outd[:, :], fin[:])

    nc.compile()
    return nc


def _host_inputs(T_F, S_F, labels):
    T_F = np.ascontiguousarray(T_F, np.float32)
    S_F = np.ascontiguousarray(S_F, np.float32)
    labels = np.asarray(labels)
    lab = np.argmax(labels, axis=-1)
    grid = (lab[None, :] == lab[:, None]).astype(np.float32)
    neg_l = 1.0 - grid
    pos_l = grid * (1.0 - np.eye(N, dtype=np.float32))
    pw = pos_l / pos_l.sum()
    nw = neg_l / neg_l.sum()
    lpw = np.full_like(pw, LOG_ZERO)
    np.log(pw, out=lpw, where=pw > 0)
    lnw = np.full_like(nw, LOG_ZERO)
    np.log(nw, out=lnw, where=nw > 0)
    mp = pos_l / pos_l.sum(-1, keepdims=True)
    mn = neg_l / neg_l.sum(-1, keepdims=True)

    rq = _rq_table()
    bq = _bq_table()
    wi = _interp_table()
    kc = np.array([[0.1], [0.02]], np.float32)

    in_maps = []
    for c in range(N_CORES):
        is_t = c < GCORES
        mat = T_F if is_t else S_F
        r0 = ROWS * (c % GCORES)
        rows = slice(r0, r0 + ROWS)
        m2 = np.zeros((2 * HALF, 2), np.float32)
        m2[:, 0 if is_t else 1] = 1.0
        pm = np.zeros((ROWS, N), np.float32)
        pm[np.arange(ROWS), r0 + np.arange(ROWS)] = 1.0
        in_maps.append(
            {
                "x": mat,
                "xr": np.ascontiguousarray(mat[rows]),
                "LP": np.ascontiguousarray(lpw[rows].astype(np.float32)),
                "LN": np.ascontiguousarray(lnw[rows].astype(np.float32)),
                "MP": np.ascontiguousarray(mp[rows].astype(np.float32)),
                "MN": np.ascontiguousarray(mn[rows].astype(np.float32)),
                "Rq": rq,
                "Bq": bq,
                "WI": wi,
                "M2": m2,
                "P": pm,
                "KC": kc,
            }
        )
    return in_maps


_NC_CACHE = {}


def run(T_F, S_F, labels, trace=False):
    if "nc" not in _NC_CACHE:
        _NC_CACHE["nc"] = build_nc()
    nc = _NC_CACHE["nc"]
    in_maps = _host_inputs(T_F, S_F, labels)
    res = run_bass_kernel_spmd(
        nc, in_maps, core_ids=list(range(N_CORES)), trace=trace
    )
    val = np.float32(res.results[0]["out"][0, 0])
    return val, res


def kernel(T_F, S_F, labels):
    val, _ = run(T_F, S_F, labels)
    return np.array(val, dtype=np.float32)
